# revision 1
# baseline (speedup 1.0000x reference)
"""Llama4 MoE experts (grouped GEMM + SwiGLU) on 8 Trainium2 NeuronCores.

Expert-parallel: core e computes expert e's token block
  Y_e = (silu(X_e @ Wg_e) * (X_e @ Wu_e)) @ Wd_e
with X_e = hidden_states[e*1024:(e+1)*1024]. No collectives needed.

All matmuls run on the PE in fp32r (full-rate fp32 mode, inputs rounded
on-chip by DVE casts). Per-core dataflow (transposed activations):
  1. PE-transpose X into Xt (H on partitions, tokens free), fp32r.
  2. MM1: gup^T = Wgu_chunk.T @ Xt accumulated over H in PSUM;
     SwiGLU (sigmoid on ScalarE + two DVE muls) -> act^T fp32r.
  3. MM2: Y = act^T_slice.T @ Wd_chunk accumulated over D in PSUM;
     eviction on ScalarE, DMA out.
Tokens go in two halves of 512 so the Xt/act slabs fit in SBUF.
"""
from contextlib import ExitStack

import numpy as np

import concourse.bass as bass
import concourse.tile as tile
from concourse import bacc, mybir
from concourse.bass_utils import run_bass_kernel_spmd
from concourse.masks import make_identity

P = 128
F32 = mybir.dt.float32
F32R = mybir.dt.float32r
SIGMOID = mybir.ActivationFunctionType.Sigmoid
COPY = mybir.ActivationFunctionType.Copy

E = 8            # experts == cores
T = 1024         # tokens per expert
H = 4096         # hidden
D = 4096         # expert (intermediate) dim

_cached_nc = None


def _build_program(T=T, H=H, D=D, TH=512, MG=4, NW=512, w_bufs=6):
    halves = T // TH
    KH = H // P
    KD = D // P
    TT = TH // P
    GG = D // (MG * P)
    NH = H // NW
    XC = min(H, 1024)
    NXC = H // XC

    nc = bacc.Bacc("TRN2", target_bir_lowering=False, debug=False)
    x_d = nc.dram_tensor("x", [T, H], F32, kind="ExternalInput").ap()
    wgu_d = nc.dram_tensor("wgu", [H, 2 * D], F32, kind="ExternalInput").ap()
    wd_d = nc.dram_tensor("wd", [D, H], F32, kind="ExternalInput").ap()
    y_d = nc.dram_tensor("y", [T, H], F32, kind="ExternalOutput").ap()

    with tile.TileContext(nc) as tc, ExitStack() as ctx:
        const = ctx.enter_context(tc.tile_pool(name="const", bufs=1))
        ident = const.tile([P, P], F32)
        make_identity(nc, ident)

        slab = ctx.enter_context(tc.tile_pool(name="slab", bufs=1))
        xt = slab.tile([P, KH * TH], F32R, tag="xt")
        act = slab.tile([P, KD * TH], F32R, tag="act")

        xstage = ctx.enter_context(tc.tile_pool(name="xstage", bufs=5))
        wstage = ctx.enter_context(tc.tile_pool(name="wstage", bufs=w_bufs))
        wr = ctx.enter_context(tc.tile_pool(name="wr", bufs=w_bufs))
        stmp = ctx.enter_context(tc.tile_pool(name="stmp", bufs=2))
        yout = ctx.enter_context(tc.tile_pool(name="yout", bufs=2))
        ps = ctx.enter_context(tc.tile_pool(name="ps", bufs=8, space="PSUM"))

        for h in range(halves):
            t0 = h * TH
            # ---- transpose X half into xt (PE transpose via identity) ----
            for tt in range(TT):
                for hc in range(NXC):
                    xs = xstage.tile([P, XC], F32, name="xs")
                    nc.sync.dma_start(
                        xs[:],
                        x_d[t0 + tt * P:t0 + (tt + 1) * P, hc * XC:(hc + 1) * XC])
                    for kk in range(XC // P):
                        k = hc * (XC // P) + kk
                        pst = ps.tile([P, P], F32, tag="ps", name="pst")
                        nc.tensor.matmul(pst[:], xs[:, kk * P:(kk + 1) * P],
                                         ident[:], is_transpose=True)
                        nc.vector.tensor_copy(
                            xt[:, k * TH + tt * P:k * TH + (tt + 1) * P], pst[:])

            # ---- MM1 (gate/up) + SwiGLU ----
            for gg in range(GG):
                psg, psu = [], []
                for which, lst in ((0, psg), (1, psu)):
                    col0 = which * D + gg * MG * P
                    for m in range(MG):
                        lst.append(ps.tile([P, TH], F32, tag="ps", name="psgu"))
                    for k in range(KH):
                        wc = wstage.tile([P, MG * P], F32, tag="wc", name="wc")
                        nc.gpsimd.dma_start(
                            wc[:], wgu_d[k * P:(k + 1) * P, col0:col0 + MG * P])
                        wrt = wr.tile([P, MG * P], F32R, tag="wrt", name="wrt")
                        nc.vector.tensor_copy(wrt[:], wc[:])
                        for m in range(MG):
                            nc.tensor.matmul(
                                lst[m][:], wrt[:, m * P:(m + 1) * P],
                                xt[:, k * TH:(k + 1) * TH],
                                start=(k == 0), stop=(k == KH - 1))
                for m in range(MG):
                    st = stmp.tile([P, TH], F32, name="st")
                    nc.scalar.activation(st[:], psg[m][:], SIGMOID)
                    gt = stmp.tile([P, TH], F32, tag="gt", name="gt")
                    nc.vector.tensor_mul(gt[:], psg[m][:], st[:])
                    d_tile = gg * MG + m
                    nc.vector.tensor_mul(
                        act[:, d_tile * TH:(d_tile + 1) * TH], psu[m][:], gt[:])

            # ---- MM2 (down projection) ----
            for nh in range(NH):
                psy = [ps.tile([P, NW], F32, tag="ps", name="psy")
                       for _ in range(TT)]
                for kd in range(KD):
                    wc = wstage.tile([P, NW], F32, tag="wc", name="wc")
                    nc.gpsimd.dma_start(
                        wc[:], wd_d[kd * P:(kd + 1) * P, nh * NW:(nh + 1) * NW])
                    wrt = wr.tile([P, NW], F32R, tag="wrt", name="wrt")
                    nc.vector.tensor_copy(wrt[:], wc[:])
                    for mt in range(TT):
                        nc.tensor.matmul(
                            psy[mt][:],
                            act[:, kd * TH + mt * P:kd * TH + (mt + 1) * P],
                            wrt[:], start=(kd == 0), stop=(kd == KD - 1))
                for mt in range(TT):
                    yo = yout.tile([P, NW], F32, name="yo")
                    nc.scalar.activation(yo[:], psy[mt][:], COPY)
                    nc.sync.dma_start(
                        y_d[t0 + mt * P:t0 + (mt + 1) * P, nh * NW:(nh + 1) * NW],
                        yo[:])

    nc.compile()
    return nc


def get_program():
    global _cached_nc
    if _cached_nc is None:
        _cached_nc = _build_program()
    return _cached_nc


def kernel(hidden_states, gate_up_proj, down_proj, run_index=None, _trace=False):
    hs = np.ascontiguousarray(np.asarray(hidden_states, dtype=np.float32))
    wgu = np.ascontiguousarray(np.asarray(gate_up_proj, dtype=np.float32))
    wd = np.ascontiguousarray(np.asarray(down_proj, dtype=np.float32))
    assert hs.shape == (E * T, H) and wgu.shape == (E, H, 2 * D) \
        and wd.shape == (E, D, H)

    nc = get_program()
    in_maps = [{"x": hs[e * T:(e + 1) * T], "wgu": wgu[e], "wd": wd[e]}
               for e in range(E)]
    res = run_bass_kernel_spmd(nc, in_maps, core_ids=list(range(E)),
                               trace=_trace)
    out = np.empty((E * T, H), dtype=np.float32)
    for e in range(E):
        out[e * T:(e + 1) * T] = res.results[e]["y"]
    if _trace:
        kernel.last_result = res
    return out

